# revision 8
# baseline (speedup 1.0000x reference)
"""BERT self-attention on 8 Trainium2 NeuronCores.

Sharding: data-parallel over batch (B=8 -> 1 batch element per core).
Every core runs the same single-core Bass kernel on its own batch slice;
weights/mask are replicated. The final output is a host-side stack.

Per-core algorithm (S=1024, HID=1024, NH=16, HD=64), all matmuls bf16
with fp32 PSUM accumulation:

  xT = X^T (host-transposed, bf16)             [HID, S]
  Q^T = Wq^T @ X^T   (lhsT = Wq natural)       [HID, S]  (+bq per-partition)
  K^T = Wk^T @ X^T                             [HID, S]  (+bk per-partition)
  V   = X @ Wv       (lhsT = xT)               [S, HID]  (+bv broadcast; see below)
  per head h:
    S^T = K_h @ Q_h^T            (scoresT: [k, q], contraction over HD=64,
                                  2 heads packed in the PE array via row tiling)
    P^T = exp(S^T/8 + mask[k])   (ScalarE, mask is per-partition in this layout,
                                  max-subtraction skipped: |scores/8| <~ 4)
    ctx = P^T.T @ [V_h + bv | 1] (lhsT = P^T directly, no transposes anywhere;
                                  the ones column yields the softmax denominator Z)
    out[:, h] = ctx[:, :64] * (1/Z)   (== softmax(S) @ (V+bv) = attn + bv)
"""

import functools

import numpy as np
import ml_dtypes

B, S, HID = 8, 1024, 1024
NH, HD = 16, 64
P = 128
NCH = HID // P  # hid chunks (8)
NKT = S // P  # key tiles (8)
NQT = S // P  # query tiles (8)
VROW = NH * (HD + 1)  # 1040: per-seq-chunk V row: 16 x (64 V cols + ones col)
N_CORES = 8

SCALE = 1.0 / float(np.sqrt(HD))


@functools.lru_cache(maxsize=None)
def _build(has_bv: bool):
    import concourse.bass as bass
    import concourse.tile as tile
    from concourse import bacc, mybir
    from contextlib import ExitStack

    fp32 = mybir.dt.float32
    bf16 = mybir.dt.bfloat16
    EXP = mybir.ActivationFunctionType.Exp

    nc = bacc.Bacc("TRN2", target_bir_lowering=False)

    xT = nc.dram_tensor("xT", [HID, S], bf16, kind="ExternalInput")
    wq = nc.dram_tensor("wq", [HID, HID], bf16, kind="ExternalInput")
    wk = nc.dram_tensor("wk", [HID, HID], bf16, kind="ExternalInput")
    wv = nc.dram_tensor("wv", [HID, HID], bf16, kind="ExternalInput")
    bq = nc.dram_tensor("bq", [P, NCH], fp32, kind="ExternalInput")
    bk = nc.dram_tensor("bk", [P, NCH], fp32, kind="ExternalInput")
    bv = nc.dram_tensor("bv", [HID], fp32, kind="ExternalInput") if has_bv else None
    mask = nc.dram_tensor("mask", [P, NKT], fp32, kind="ExternalInput")
    out = nc.dram_tensor("out", [S, HID], fp32, kind="ExternalOutput")

    with tile.TileContext(nc) as tc, ExitStack() as ctx:
        persist = ctx.enter_context(tc.tile_pool(name="persist", bufs=1))
        misc = ctx.enter_context(tc.tile_pool(name="misc", bufs=8))
        pT_pool = ctx.enter_context(tc.tile_pool(name="pT", bufs=2))
        out_pool = ctx.enter_context(tc.tile_pool(name="out", bufs=2))
        qkv_ps = ctx.enter_context(tc.tile_pool(name="qkv_ps", bufs=2, space="PSUM"))
        sc_ps = ctx.enter_context(tc.tile_pool(name="sc_ps", bufs=2, space="PSUM"))
        cx_ps = ctx.enter_context(tc.tile_pool(name="cx_ps", bufs=2, space="PSUM"))

        # ---- persistent SBUF tensors ----
        xT_sb = persist.tile([P, NCH, S], bf16)  # [p, hid_chunk, seq]
        wq_sb = persist.tile([P, NCH, HID], bf16)  # [p, hidin_chunk, hid_out]
        wk_sb = persist.tile([P, NCH, HID], bf16)
        wv_sb = persist.tile([P, NCH, HID], bf16)
        qT_sb = persist.tile([P, NCH, S], bf16)  # [p, hidout_chunk, seq]
        kT_sb = persist.tile([P, NCH, S], bf16)
        v_sb = persist.tile([P, NKT, VROW], bf16)  # [p(seq), seq_chunk, 16*(64+1)]
        bq_sb = persist.tile([P, NCH], fp32)
        bk_sb = persist.tile([P, NCH], fp32)
        mask_sb = persist.tile([P, NKT], fp32)
        bv_sb = persist.tile([P, HID], fp32) if has_bv else None

        # ---- input DMAs ----
        nc.sync.dma_start(out=bq_sb, in_=bq[:, :])
        nc.sync.dma_start(out=bk_sb, in_=bk[:, :])
        nc.sync.dma_start(out=mask_sb, in_=mask[:, :])
        if has_bv:
            # broadcast bv[HID] across all 128 partitions
            bv_bcast = bass.AP(tensor=bv.tensor if hasattr(bv, "tensor") else bv,
                               offset=0, ap=[[0, P], [1, HID]])
            nc.sync.dma_start(out=bv_sb, in_=bv_bcast)
        for c in range(NCH):
            nc.sync.dma_start(out=xT_sb[:, c, :], in_=xT[c * P:(c + 1) * P, :])
        for w_sb, w in ((wv_sb, wv), (wq_sb, wq), (wk_sb, wk)):
            for c in range(NCH):
                nc.sync.dma_start(out=w_sb[:, c, :], in_=w[c * P:(c + 1) * P, :])

        # ones columns for the softmax denominator live at col 64 of each
        # 65-wide head block; V copies below only overwrite cols 0..63
        nc.vector.memset(v_sb, 1.0)

        # ---- V = X @ Wv  (+bv), stored [seq, head-interleaved 65] ----
        for st in range(NKT):  # seq chunk
            for half in range(2):
                ps = qkv_ps.tile([P, 512], fp32, name="qkv_psum")
                for kc in range(NCH):
                    nc.tensor.matmul(
                        ps,
                        lhsT=xT_sb[:, kc, st * P:(st + 1) * P],
                        rhs=wv_sb[:, kc, half * 512:(half + 1) * 512],
                        start=(kc == 0),
                        stop=(kc == NCH - 1),
                    )
                dst = (
                    v_sb[:, st, :]
                    .rearrange("p (h x) -> p h x", x=HD + 1)[:, half * 8:(half + 1) * 8, 0:HD]
                )
                src = ps.rearrange("p (h x) -> p h x", x=HD)
                if has_bv:
                    bvs = (
                        bv_sb[:, half * 512:(half + 1) * 512]
                        .rearrange("p (h x) -> p h x", x=HD)
                    )
                    nc.vector.tensor_add(out=dst, in0=src, in1=bvs)
                else:
                    nc.vector.tensor_copy(out=dst, in_=src)

        # ---- per hid_out chunk c: Q^T, K^T, then heads 2c, 2c+1 ----
        for c in range(NCH):
            for dst_sb, w_sb, b_sb in ((qT_sb, wq_sb, bq_sb), (kT_sb, wk_sb, bk_sb)):
                for half in range(2):
                    ps = qkv_ps.tile([P, 512], fp32, name="qkv_psum")
                    for kc in range(NCH):
                        nc.tensor.matmul(
                            ps,
                            lhsT=w_sb[:, kc, c * P:(c + 1) * P],
                            rhs=xT_sb[:, kc, half * 512:(half + 1) * 512],
                            start=(kc == 0),
                            stop=(kc == NCH - 1),
                        )
                    nc.vector.tensor_scalar_add(
                        out=dst_sb[:, c, half * 512:(half + 1) * 512],
                        in0=ps,
                        scalar1=b_sb[:, c:c + 1],
                    )

            # ---- attention for the two heads living in chunk c ----
            # The two heads' score matmuls use disjoint PE row groups
            # (K=64 at partition offsets 0 and 64), so emit each pair
            # back-to-back: the PE runs row-group-disjoint matmuls
            # concurrently only when they are adjacent in its queue.
            pT_tiles = [pT_pool.tile([P, NKT, S], bf16, name="pT") for _ in range(2)]
            for kt in range(NKT):
                ps_pair = [sc_ps.tile([P, S], fp32, name="score_psum") for _ in range(2)]
                for half in range(2):
                    for sub in range(2):
                        po = 64 * sub
                        nc.tensor.matmul(
                            ps_pair[sub][:, half * 512:(half + 1) * 512],
                            lhsT=kT_sb[po:po + HD, c, kt * P:(kt + 1) * P],
                            rhs=qT_sb[po:po + HD, c, half * 512:(half + 1) * 512],
                            start=True,
                            stop=True,
                            tile_position=(po, 0),
                        )
                for sub in range(2):
                    # P^T = exp(scores/8 + mask_k); bf16 out, straight to SBUF
                    nc.scalar.activation(
                        out=pT_tiles[sub][:, kt, :],
                        in_=ps_pair[sub],
                        func=EXP,
                        bias=mask_sb[:, kt:kt + 1],
                        scale=SCALE,
                    )

            pair_out = out_pool.tile([P, NQT, 2 * HD], fp32, name="pair_out")
            for sub in range(2):
                h = 2 * c + sub
                pT_h = pT_tiles[sub]
                for qt in range(NQT):
                    cps = cx_ps.tile([P, HD + 1], fp32, name="ctx_psum")
                    for kc in range(NKT):
                        nc.tensor.matmul(
                            cps,
                            lhsT=pT_h[:, kc, qt * P:(qt + 1) * P],
                            rhs=v_sb[:, kc, h * (HD + 1):(h + 1) * (HD + 1)],
                            start=(kc == 0),
                            stop=(kc == NKT - 1),
                        )
                    recip = misc.tile([P, 1], fp32, name="recip")
                    nc.vector.reciprocal(recip, cps[:, HD:HD + 1])
                    nc.vector.tensor_scalar_mul(
                        out=pair_out[:, qt, sub * HD:(sub + 1) * HD],
                        in0=cps[:, 0:HD],
                        scalar1=recip,
                    )
            # stream this head pair's output columns out while later heads run
            for qt in range(NQT):
                nc.sync.dma_start(
                    out=out[qt * P:(qt + 1) * P, c * P:(c + 1) * P],
                    in_=pair_out[:, qt, :],
                )

    nc.finalize()
    return nc


def _prep_inputs(inputs):
    bf16 = ml_dtypes.bfloat16
    hs = np.asarray(inputs["hidden_states"], dtype=np.float32)
    am = np.asarray(inputs["attention_mask"], dtype=np.float32)
    Wq = np.asarray(inputs["Wq"], dtype=np.float32)
    Wk = np.asarray(inputs["Wk"], dtype=np.float32)
    Wv = np.asarray(inputs["Wv"], dtype=np.float32)
    bq = np.asarray(inputs["bq"], dtype=np.float32)
    bk = np.asarray(inputs["bk"], dtype=np.float32)
    bv = np.asarray(inputs["bv"], dtype=np.float32)

    has_bv = bool(np.any(bv))

    wq_b = np.ascontiguousarray(Wq.astype(bf16))
    wk_b = np.ascontiguousarray(Wk.astype(bf16))
    wv_b = np.ascontiguousarray(Wv.astype(bf16))
    bq_c = np.ascontiguousarray(bq.reshape(NCH, P).T)
    bk_c = np.ascontiguousarray(bk.reshape(NCH, P).T)

    hs_b = hs.astype(bf16)
    in_maps = []
    for b in range(B):
        m = {
            "xT": np.ascontiguousarray(hs_b[b].T),
            "wq": wq_b,
            "wk": wk_b,
            "wv": wv_b,
            "bq": bq_c,
            "bk": bk_c,
            "mask": np.ascontiguousarray(am[b, 0, 0].reshape(NKT, P).T),
        }
        if has_bv:
            m["bv"] = bv
        in_maps.append(m)
    return in_maps, has_bv


def _run(inputs, trace=False):
    from concourse.bass_utils import run_bass_kernel_spmd

    in_maps, has_bv = _prep_inputs(inputs)
    nc = _build(has_bv)
    res = run_bass_kernel_spmd(
        nc, in_maps, core_ids=list(range(N_CORES)), trace=trace
    )
    out = np.stack([np.asarray(r["out"], dtype=np.float32) for r in res.results])
    return out, res


def kernel(**inputs) -> np.ndarray:
    out, _ = _run(inputs, trace=False)
    return out


# revision 11
# speedup vs baseline: 1.2565x; 1.2565x over previous
"""BERT self-attention on 8 Trainium2 NeuronCores.

Sharding: data-parallel over batch (B=8 -> 1 batch element per core).
Every core runs the same single-core Bass kernel on its own batch slice;
weights/mask are replicated. The final output is a host-side stack.

Per-core algorithm (S=1024, HID=1024, NH=16, HD=64), all matmuls bf16
with fp32 PSUM accumulation:

  xT = X^T (host-transposed, bf16)             [HID, S]
  Q^T = Wq^T @ X^T   (lhsT = Wq natural)       [HID, S]  (+bq per-partition)
  K^T = Wk^T @ X^T                             [HID, S]  (+bk per-partition)
  V   = X @ Wv       (lhsT = xT)               [S, HID]  (+bv broadcast; see below)
  per head h:
    S^T = K_h @ Q_h^T            (scoresT: [k, q], contraction over HD=64,
                                  2 heads packed in the PE array via row tiling)
    P^T = exp(S^T/8 + mask[k])   (ScalarE, mask is per-partition in this layout,
                                  max-subtraction skipped: |scores/8| <~ 4)
    ctx = P^T.T @ [V_h + bv | 1] (lhsT = P^T directly, no transposes anywhere;
                                  the ones column yields the softmax denominator Z)
    out[:, h] = ctx[:, :64] * (1/Z)   (== softmax(S) @ (V+bv) = attn + bv)
"""

import functools

import numpy as np
import ml_dtypes

B, S, HID = 8, 1024, 1024
NH, HD = 16, 64
P = 128
NCH = HID // P  # hid chunks (8)
NKT = S // P  # key tiles (8)
NQT = S // P  # query tiles (8)
VROW = NH * (HD + 1)  # 1040: per-seq-chunk V row: 16 x (64 V cols + ones col)
N_CORES = 8

SCALE = 1.0 / float(np.sqrt(HD))


@functools.lru_cache(maxsize=None)
def _build(has_bv: bool):
    import concourse.bass as bass
    import concourse.tile as tile
    from concourse import bacc, mybir
    from contextlib import ExitStack

    fp32 = mybir.dt.float32
    bf16 = mybir.dt.bfloat16
    EXP = mybir.ActivationFunctionType.Exp

    nc = bacc.Bacc("TRN2", target_bir_lowering=False)

    xT = nc.dram_tensor("xT", [HID, S], bf16, kind="ExternalInput")
    wq = nc.dram_tensor("wq", [HID, HID], bf16, kind="ExternalInput")
    wk = nc.dram_tensor("wk", [HID, HID], bf16, kind="ExternalInput")
    wv = nc.dram_tensor("wv", [HID, HID], bf16, kind="ExternalInput")
    bq = nc.dram_tensor("bq", [P, NCH], fp32, kind="ExternalInput")
    bk = nc.dram_tensor("bk", [P, NCH], fp32, kind="ExternalInput")
    bv = nc.dram_tensor("bv", [HID], fp32, kind="ExternalInput") if has_bv else None
    mask = nc.dram_tensor("mask", [P, NKT], fp32, kind="ExternalInput")
    out = nc.dram_tensor("out", [S, HID], fp32, kind="ExternalOutput")

    with tile.TileContext(nc) as tc, ExitStack() as ctx:
        persist = ctx.enter_context(tc.tile_pool(name="persist", bufs=1))
        misc = ctx.enter_context(tc.tile_pool(name="misc", bufs=8))
        pT_pool = ctx.enter_context(tc.tile_pool(name="pT", bufs=2))
        out_pool = ctx.enter_context(tc.tile_pool(name="out", bufs=2))
        qkv_ps = ctx.enter_context(tc.tile_pool(name="qkv_ps", bufs=2, space="PSUM"))
        sc_ps = ctx.enter_context(tc.tile_pool(name="sc_ps", bufs=2, space="PSUM"))
        cx_ps = ctx.enter_context(tc.tile_pool(name="cx_ps", bufs=2, space="PSUM"))

        # ---- persistent SBUF tensors ----
        xT_sb = persist.tile([P, NCH, S], bf16)  # [p, hid_chunk, seq]
        wq_sb = persist.tile([P, NCH, HID], bf16)  # [p, hidin_chunk, hid_out]
        wk_sb = persist.tile([P, NCH, HID], bf16)
        wv_sb = persist.tile([P, NCH, HID], bf16)
        qT_sb = persist.tile([P, NCH, S], bf16)  # [p, hidout_chunk, seq]
        # K^T stored zero-padded to K=128 per head: variant v holds head
        # 2c+v's 64 rows at partition offset 64*v, the other half zero.
        # This keeps the score matmuls at 128 contraction rows (FWL stays
        # enabled; 64-row weight loads serialize ~100ns/matmul otherwise).
        kTp_sb = persist.tile([P, NCH, 2, S], bf16)
        v_sb = persist.tile([P, NKT, VROW], bf16)  # [p(seq), seq_chunk, 16*(64+1)]
        bq_sb = persist.tile([P, NCH], fp32)
        bk_sb = persist.tile([P, NCH], fp32)
        mask_sb = persist.tile([P, NKT], fp32)
        bv_sb = persist.tile([P, HID], fp32) if has_bv else None

        # ---- input DMAs ----
        nc.sync.dma_start(out=bq_sb, in_=bq[:, :])
        nc.sync.dma_start(out=bk_sb, in_=bk[:, :])
        nc.sync.dma_start(out=mask_sb, in_=mask[:, :])
        if has_bv:
            # broadcast bv[HID] across all 128 partitions
            bv_bcast = bass.AP(tensor=bv.tensor if hasattr(bv, "tensor") else bv,
                               offset=0, ap=[[0, P], [1, HID]])
            nc.sync.dma_start(out=bv_sb, in_=bv_bcast)
        for c in range(NCH):
            nc.sync.dma_start(out=xT_sb[:, c, :], in_=xT[c * P:(c + 1) * P, :])
        for w_sb, w in ((wv_sb, wv), (wq_sb, wq), (wk_sb, wk)):
            for c in range(NCH):
                nc.sync.dma_start(out=w_sb[:, c, :], in_=w[c * P:(c + 1) * P, :])

        # ones columns for the softmax denominator live at col 64 of each
        # 65-wide head block; V copies below only overwrite cols 0..63
        nc.vector.memset(v_sb, 1.0)
        # zero the padded K^T store on the otherwise-idle gpsimd engine;
        # the K copies later fill in only each variant's live 64 rows
        nc.gpsimd.memset(kTp_sb, 0.0)

        # ---- V = X @ Wv  (+bv), stored [seq, head-interleaved 65] ----
        for st in range(NKT):  # seq chunk
            for half in range(2):
                ps = qkv_ps.tile([P, 512], fp32, name="qkv_psum")
                for kc in range(NCH):
                    nc.tensor.matmul(
                        ps,
                        lhsT=xT_sb[:, kc, st * P:(st + 1) * P],
                        rhs=wv_sb[:, kc, half * 512:(half + 1) * 512],
                        start=(kc == 0),
                        stop=(kc == NCH - 1),
                    )
                dst = (
                    v_sb[:, st, :]
                    .rearrange("p (h x) -> p h x", x=HD + 1)[:, half * 8:(half + 1) * 8, 0:HD]
                )
                src = ps.rearrange("p (h x) -> p h x", x=HD)
                if has_bv:
                    bvs = (
                        bv_sb[:, half * 512:(half + 1) * 512]
                        .rearrange("p (h x) -> p h x", x=HD)
                    )
                    nc.vector.tensor_add(out=dst, in0=src, in1=bvs)
                else:
                    nc.vector.tensor_copy(out=dst, in_=src)

        # ---- per hid_out chunk c: Q^T, K^T, then heads 2c, 2c+1 ----
        for c in range(NCH):
            for half in range(2):
                ps = qkv_ps.tile([P, 512], fp32, name="qkv_psum")
                for kc in range(NCH):
                    nc.tensor.matmul(
                        ps,
                        lhsT=wq_sb[:, kc, c * P:(c + 1) * P],
                        rhs=xT_sb[:, kc, half * 512:(half + 1) * 512],
                        start=(kc == 0),
                        stop=(kc == NCH - 1),
                    )
                nc.vector.tensor_scalar_add(
                    out=qT_sb[:, c, half * 512:(half + 1) * 512],
                    in0=ps,
                    scalar1=bq_sb[:, c:c + 1],
                )
            for half in range(2):
                ps = qkv_ps.tile([P, 512], fp32, name="qkv_psum")
                for kc in range(NCH):
                    nc.tensor.matmul(
                        ps,
                        lhsT=wk_sb[:, kc, c * P:(c + 1) * P],
                        rhs=xT_sb[:, kc, half * 512:(half + 1) * 512],
                        start=(kc == 0),
                        stop=(kc == NCH - 1),
                    )
                for sub in range(2):  # head 2c+sub lives at partitions 64*sub..
                    po = 64 * sub
                    nc.vector.tensor_scalar_add(
                        out=kTp_sb[po:po + HD, c, sub, half * 512:(half + 1) * 512],
                        in0=ps[po:po + HD, :],
                        scalar1=bk_sb[po:po + HD, c:c + 1],
                    )

            # ---- attention for the two heads living in chunk c ----
            pT_tiles = []
            for sub in range(2):
                h = 2 * c + sub
                pT_h = pT_pool.tile([P, NKT, S], bf16, name="pT")
                pT_tiles.append(pT_h)
                for kt in range(NKT):
                    ps = sc_ps.tile([P, S], fp32, name="score_psum")
                    for half in range(2):
                        nc.tensor.matmul(
                            ps[:, half * 512:(half + 1) * 512],
                            lhsT=kTp_sb[:, c, sub, kt * P:(kt + 1) * P],
                            rhs=qT_sb[:, c, half * 512:(half + 1) * 512],
                            start=True,
                            stop=True,
                        )
                    # P^T = exp(scores/8 + mask_k); bf16 out, straight to SBUF
                    nc.scalar.activation(
                        out=pT_h[:, kt, :],
                        in_=ps,
                        func=EXP,
                        bias=mask_sb[:, kt:kt + 1],
                        scale=SCALE,
                    )

            pair_out = out_pool.tile([P, NQT, 2 * HD], fp32, name="pair_out")
            for sub in range(2):
                h = 2 * c + sub
                pT_h = pT_tiles[sub]
                for qt in range(NQT):
                    cps = cx_ps.tile([P, HD + 1], fp32, name="ctx_psum")
                    for kc in range(NKT):
                        nc.tensor.matmul(
                            cps,
                            lhsT=pT_h[:, kc, qt * P:(qt + 1) * P],
                            rhs=v_sb[:, kc, h * (HD + 1):(h + 1) * (HD + 1)],
                            start=(kc == 0),
                            stop=(kc == NKT - 1),
                        )
                    recip = misc.tile([P, 1], fp32, name="recip")
                    nc.vector.reciprocal(recip, cps[:, HD:HD + 1])
                    nc.vector.tensor_scalar_mul(
                        out=pair_out[:, qt, sub * HD:(sub + 1) * HD],
                        in0=cps[:, 0:HD],
                        scalar1=recip,
                    )
            # stream this head pair's output columns out while later heads run
            for qt in range(NQT):
                nc.sync.dma_start(
                    out=out[qt * P:(qt + 1) * P, c * P:(c + 1) * P],
                    in_=pair_out[:, qt, :],
                )

    nc.finalize()
    return nc


def _prep_inputs(inputs):
    bf16 = ml_dtypes.bfloat16
    hs = np.asarray(inputs["hidden_states"], dtype=np.float32)
    am = np.asarray(inputs["attention_mask"], dtype=np.float32)
    Wq = np.asarray(inputs["Wq"], dtype=np.float32)
    Wk = np.asarray(inputs["Wk"], dtype=np.float32)
    Wv = np.asarray(inputs["Wv"], dtype=np.float32)
    bq = np.asarray(inputs["bq"], dtype=np.float32)
    bk = np.asarray(inputs["bk"], dtype=np.float32)
    bv = np.asarray(inputs["bv"], dtype=np.float32)

    has_bv = bool(np.any(bv))

    wq_b = np.ascontiguousarray(Wq.astype(bf16))
    wk_b = np.ascontiguousarray(Wk.astype(bf16))
    wv_b = np.ascontiguousarray(Wv.astype(bf16))
    bq_c = np.ascontiguousarray(bq.reshape(NCH, P).T)
    bk_c = np.ascontiguousarray(bk.reshape(NCH, P).T)

    hs_b = hs.astype(bf16)
    in_maps = []
    for b in range(B):
        m = {
            "xT": np.ascontiguousarray(hs_b[b].T),
            "wq": wq_b,
            "wk": wk_b,
            "wv": wv_b,
            "bq": bq_c,
            "bk": bk_c,
            "mask": np.ascontiguousarray(am[b, 0, 0].reshape(NKT, P).T),
        }
        if has_bv:
            m["bv"] = bv
        in_maps.append(m)
    return in_maps, has_bv


def _run(inputs, trace=False):
    from concourse.bass_utils import run_bass_kernel_spmd

    in_maps, has_bv = _prep_inputs(inputs)
    nc = _build(has_bv)
    res = run_bass_kernel_spmd(
        nc, in_maps, core_ids=list(range(N_CORES)), trace=trace
    )
    out = np.stack([np.asarray(r["out"], dtype=np.float32) for r in res.results])
    return out, res


def kernel(**inputs) -> np.ndarray:
    out, _ = _run(inputs, trace=False)
    return out
